# revision 1
# baseline (speedup 1.0000x reference)
"""ClusterGCN 2-layer kernel for 8 Trainium2 NeuronCores (Bass/Tile).

Strategy (graph/data parallel, per the sharding hint):
  - Target nodes sharded 8 ways (12500/core, padded to 12544 = 98*128 tiles).
  - Each core owns the edges whose target (col) is in its shard, grouped by
    128-node target tile, with self loops added as explicit edges. Edge
    counts are padded to be uniform across cores (single SPMD program).
  - Feature gathers use the MoE dma_gather primitive (int16 indices), so the
    gather source is split into 4 chunks by src%4; each supertile of ST
    target tiles issues one big gather per chunk. Blocks of 128 edge slots
    are target-tile-pure so each block feeds one one-hot segment-sum matmul.
  - Layer 1 accumulates agg.T = sum_e x[src_e] (x) onehot(col_e) in PSUM
    [feat, tgt], applies deg_inv via a rank-1 broadcast, then the out/root
    weights + bias + ReLU, leaving hT resident in SBUF.
  - z2 = relu(h) @ W2_out.T per shard, AllGathered (only collective).
  - Layer 2 gathers z2 rows (same edge structure), accumulates [tgt, 64],
    applies deg_inv / root / bias, writes the output shard.

All index tables / transposed weights / iota / identity are precomputed on
the host and passed as extra per-core inputs.
"""
import math
import numpy as np

P = 128
NCH = 4          # gather-source chunks (int16 index limit)
ST = 8           # target tiles per gather supertile


class Cfg:
    def __init__(self, n=100000, e=1600000, cores=8, c_in=128, c_hid=128,
                 c_out=64, st=ST):
        self.N, self.E, self.CORES = n, e, cores
        self.C_IN, self.C_HID, self.C_OUT = c_in, c_hid, c_out
        self.NS = n // cores                      # nodes per shard
        assert self.NS * cores == n
        assert self.NS % NCH == 0
        self.T = math.ceil(self.NS / P)           # target tiles per core
        self.NSP = self.T * P                     # padded shard size
        assert self.NSP % NCH == 0
        assert n % NCH == 0
        self.ST = st
        self.NST = math.ceil(self.T / st)         # supertiles


class Plan:
    """Static (cross-core-uniform) edge layout."""
    def __init__(self, cfg, K4):
        self.K4 = K4                              # [T][NCH] blocks
        self.Ktot = [sum(K4[t]) for t in range(cfg.T)]
        self.KSUM = sum(self.Ktot)
        self.toff = np.concatenate([[0], np.cumsum(self.Ktot)]).astype(int)
        # supertile structure
        self.sts = [list(range(s * cfg.ST, min((s + 1) * cfg.ST, cfg.T)))
                    for s in range(cfg.NST)]
        # per (st, ch): NI and block base per tile
        self.NI = [[sum(K4[t][ch] for t in tiles) * P for ch in range(NCH)]
                   for tiles in self.sts]
        self.base_blk = []
        for tiles in self.sts:
            bb = {}
            for ch in range(NCH):
                acc = 0
                for t in tiles:
                    bb[(t, ch)] = acc
                    acc += K4[t][ch]
            self.base_blk.append(bb)
        # idx tensor column offsets per (st, ch), in int16 columns (NI/16)
        w = []
        for s in range(len(self.sts)):
            for ch in range(NCH):
                w.append(self.NI[s][ch] // 16)
        self.woff = np.concatenate([[0], np.cumsum(w)]).astype(int)
        self.WTOT = int(self.woff[-1])

    def wslice(self, s, ch):
        i = s * NCH + ch
        return int(self.woff[i]), int(self.woff[i + 1])


def wrap_idxs(flat):
    """[NI] int -> [128, NI//16] int16: j -> (j%16, j//16), replicated x8."""
    ni = flat.shape[0]
    w = flat.reshape(ni // 16, 16).T.astype(np.int16)
    return np.tile(w, (8, 1))


def preprocess(cfg, x, edge_index, W1_out, b1_out, W1_root, W2_out, b2_out,
               W2_root):
    N, NS, NSP, T = cfg.N, cfg.NS, cfg.NSP, cfg.T

    row = np.asarray(edge_index[0], dtype=np.int64)
    col = np.asarray(edge_index[1], dtype=np.int64)
    keep = row != col
    r = row[keep].astype(np.int64)
    c = col[keep].astype(np.int64)

    deg = np.bincount(c, minlength=N).astype(np.float32) + 1.0
    deg_inv = (1.0 / deg).astype(np.float32)

    selfs = np.arange(N, dtype=np.int64)
    r = np.concatenate([r, selfs])
    c = np.concatenate([c, selfs])

    shard = c // NS
    lt = c - shard * NS
    tt = lt // P
    cl = lt % P
    ch = r % NCH

    # per (core, tile, chunk) counts -> uniform K4
    flat_key = (shard * T + tt) * NCH + ch
    cnt = np.bincount(flat_key, minlength=cfg.CORES * T * NCH
                      ).reshape(cfg.CORES, T, NCH)
    K4 = [[int(math.ceil(cnt[:, t, q].max() / P)) for q in range(NCH)]
          for t in range(T)]
    plan = Plan(cfg, K4)

    order = np.lexsort((ch, tt, shard))
    r_s, tt_s, cl_s, ch_s = r[order], tt[order], cl[order], ch[order]
    key_s = (shard[order] * T + tt_s) * NCH + ch_s
    seg = np.searchsorted(key_s, np.arange(cfg.CORES * T * NCH + 1))

    iota = np.broadcast_to(np.arange(P, dtype=np.float32), (P, P)).copy()
    ident = np.eye(P, dtype=np.float32)
    ones = np.ones((1, P), np.float32)
    w1o = np.ascontiguousarray(np.asarray(W1_out, np.float32).T)
    w1r = np.ascontiguousarray(np.asarray(W1_root, np.float32).T)
    w2o = np.ascontiguousarray(np.asarray(W2_out, np.float32).T)
    w2r = np.ascontiguousarray(np.asarray(W2_root, np.float32).T)
    b1c = np.asarray(b1_out, np.float32).reshape(-1, 1)
    b2r = np.asarray(b2_out, np.float32).reshape(1, -1)
    xf = np.ascontiguousarray(np.asarray(x, np.float32))

    in_maps, emus = [], []
    for cc in range(cfg.CORES):
        # per-(t,ch) slot arrays: src global id (pad src=chunk row 0 i.e.
        # global ch), col (-1 pad)
        src_slots = np.zeros((T, max(plan.Ktot) if T else 0), object)
        colv = np.full((P, plan.KSUM), -1.0, np.float32)
        # per-slot global src, laid out [p, column] like colv
        srcg = np.zeros((P, plan.KSUM), np.int64)
        for t in range(T):
            coff = int(plan.toff[t])
            ccol = 0
            for q in range(NCH):
                i0 = seg[(cc * T + t) * NCH + q]
                i1 = seg[(cc * T + t) * NCH + q + 1]
                K = K4[t][q]
                L = i1 - i0
                s_pad = np.full(K * P, q, np.int64)    # pad: chunk row 0
                c_pad = np.full(K * P, -1.0, np.float32)
                s_pad[:L] = r_s[i0:i1]
                c_pad[:L] = cl_s[i0:i1].astype(np.float32)
                srcg[:, coff + ccol:coff + ccol + K] = \
                    s_pad.reshape(K, P).T
                colv[:, coff + ccol:coff + ccol + K] = \
                    c_pad.reshape(K, P).T
                ccol += K
        # build wrapped idx tensors per (st, ch) call
        zrow = (srcg // NS) * NSP + (srcg % NS)       # padded z2 row
        idx1w = np.zeros((P, plan.WTOT), np.int16)
        idx2w = np.zeros((P, plan.WTOT), np.int16)
        for s, tiles in enumerate(plan.sts):
            for q in range(NCH):
                w0, w1 = plan.wslice(s, q)
                if w1 == w0:
                    continue
                flat1 = np.zeros((w1 - w0) * 16, np.int64)
                flat2 = np.zeros((w1 - w0) * 16, np.int64)
                pos = 0
                for t in tiles:
                    coff = int(plan.toff[t])
                    cbase = sum(K4[t][qq] for qq in range(q))
                    K = K4[t][q]
                    for k in range(K):
                        col_i = coff + cbase + k
                        flat1[pos:pos + P] = srcg[:, col_i] // NCH
                        flat2[pos:pos + P] = zrow[:, col_i] // NCH
                        pos += P
                assert pos == (w1 - w0) * 16
                idx1w[:, w0:w1] = wrap_idxs(flat1)
                idx2w[:, w0:w1] = wrap_idxs(flat2)

        xloc = np.zeros((NSP, cfg.C_IN), np.float32)
        xloc[:NS] = xf[cc * NS:(cc + 1) * NS]
        dloc = np.ones(NSP, np.float32)
        dloc[:NS] = deg_inv[cc * NS:(cc + 1) * NS]
        dcol = np.ascontiguousarray(dloc.reshape(T, P).T)
        drow = dloc.reshape(1, NSP)

        in_maps.append({
            "xsrc": xf, "xloc": xloc,
            "idx1w": idx1w, "idx2w": idx2w, "colv": colv,
            "dcol": dcol, "drow": drow,
            "w1o": w1o, "w1r": w1r, "w2o": w2o, "w2r": w2r,
            "b1c": b1c, "b2r": b2r, "ones": ones, "iota": iota,
            "ident": ident,
        })
        emus.append({"srcg": srcg, "zrow": zrow})
    return in_maps, plan, emus


def build_program(cfg, plan):
    import concourse.bass as bass  # noqa: F401
    import concourse.bacc as bacc
    import concourse.mybir as mybir
    import concourse.tile as tile

    FP = mybir.dt.float32
    I16 = mybir.dt.int16
    AF = mybir.ActivationFunctionType
    OP = mybir.AluOpType
    N, NSP, T = cfg.N, cfg.NSP, cfg.T
    CI, CH_, CO = cfg.C_IN, cfg.C_HID, cfg.C_OUT
    K4, KSUM = plan.K4, plan.KSUM
    NI_max = max(max(r) for r in plan.NI)

    nc = bacc.Bacc("TRN2", target_bir_lowering=False, debug=False,
                   num_devices=cfg.CORES)

    xsrc = nc.dram_tensor("xsrc", [N, CI], FP, kind="ExternalInput")
    xloc = nc.dram_tensor("xloc", [NSP, CI], FP, kind="ExternalInput")
    idx1w = nc.dram_tensor("idx1w", [P, plan.WTOT], I16, kind="ExternalInput")
    idx2w = nc.dram_tensor("idx2w", [P, plan.WTOT], I16, kind="ExternalInput")
    colv = nc.dram_tensor("colv", [P, KSUM], FP, kind="ExternalInput")
    dcol = nc.dram_tensor("dcol", [P, T], FP, kind="ExternalInput")
    drow = nc.dram_tensor("drow", [1, NSP], FP, kind="ExternalInput")
    w1o = nc.dram_tensor("w1o", [CI, CH_], FP, kind="ExternalInput")
    w1r = nc.dram_tensor("w1r", [CI, CH_], FP, kind="ExternalInput")
    w2o = nc.dram_tensor("w2o", [CH_, CO], FP, kind="ExternalInput")
    w2r = nc.dram_tensor("w2r", [CH_, CO], FP, kind="ExternalInput")
    b1c = nc.dram_tensor("b1c", [CH_, 1], FP, kind="ExternalInput")
    b2r = nc.dram_tensor("b2r", [1, CO], FP, kind="ExternalInput")
    ones = nc.dram_tensor("ones", [1, P], FP, kind="ExternalInput")
    iota = nc.dram_tensor("iota", [P, P], FP, kind="ExternalInput")
    ident = nc.dram_tensor("ident", [P, P], FP, kind="ExternalInput")

    out = nc.dram_tensor("out", [NSP, CO], FP, kind="ExternalOutput")
    z2l = nc.dram_tensor("z2l", [NSP, CO], FP)
    z2f = nc.dram_tensor("z2f", [cfg.CORES * NSP, CO], FP, addr_space="Shared")

    with tile.TileContext(nc) as tc:
        with (
            tc.tile_pool(name="cst", bufs=1) as cst,
            tc.tile_pool(name="hp", bufs=1) as hp,
            tc.tile_pool(name="gp", bufs=2) as gp,
            tc.tile_pool(name="ip", bufs=3) as ip,
            tc.tile_pool(name="sp", bufs=3) as sp,
            tc.tile_pool(name="xp", bufs=3) as xp,
            tc.tile_pool(name="wk", bufs=3) as wk,
            tc.tile_pool(name="ps_scat", bufs=3, space="PSUM") as ps_scat,
            tc.tile_pool(name="ps_mm", bufs=2, space="PSUM") as ps_mm,
            tc.tile_pool(name="ps_aux", bufs=3, space="PSUM") as ps_aux,
        ):
            def load_const(t_dram, shape, dtype=FP):
                t_sb = cst.tile(shape, dtype, tag=t_dram.name)
                nc.sync.dma_start(out=t_sb[:], in_=t_dram[:, :])
                return t_sb

            colv_sb = load_const(colv, [P, KSUM])
            dcol_sb = load_const(dcol, [P, T])
            drow_sb = load_const(drow, [1, NSP])
            w1o_sb = load_const(w1o, [CI, CH_])
            w1r_sb = load_const(w1r, [CI, CH_])
            w2o_sb = load_const(w2o, [CH_, CO])
            w2r_sb = load_const(w2r, [CH_, CO])
            b1c_sb = load_const(b1c, [CH_, 1])
            b2r_sb = load_const(b2r, [1, CO])
            ones_sb = load_const(ones, [1, P])
            iota_sb = load_const(iota, [P, P])
            ident_sb = load_const(ident, [P, P])

            hT = hp.tile([P, NSP], FP)

            def build_onehot_tc(t, q):
                """One-hot blocks for (tile t, chunk q): [P, K4*P]."""
                K = K4[t][q]
                cbase = sum(K4[t][qq] for qq in range(q))
                o = int(plan.toff[t]) + cbase
                s = sp.tile([P, K * P], FP, tag="s")
                nc.vector.tensor_tensor(
                    out=s[:].rearrange("p (k j) -> p k j", k=K),
                    in0=iota_sb[:].unsqueeze(1).to_broadcast([P, K, P]),
                    in1=colv_sb[:, o:o + K].unsqueeze(2).to_broadcast(
                        [P, K, P]),
                    op=OP.is_equal,
                )
                return s

            def gather_supertile(idxw_dram, table_ap, elem, estep, s, q, tag):
                NI = plan.NI[s][q]
                if NI == 0:
                    return None
                w0, w1 = plan.wslice(s, q)
                it = ip.tile([P, w1 - w0], I16, tag=tag + "i")
                nc.sync.dma_start(out=it[:], in_=idxw_dram[:, w0:w1])
                g = gp.tile([P, (NI // P) * elem], FP, tag=tag)
                nc.gpsimd.dma_gather(
                    out_ap=g[:].rearrange("p (k j) -> p k j", k=NI // P),
                    in_ap=table_ap,
                    idxs_ap=it[:],
                    num_idxs=NI,
                    num_idxs_reg=NI,
                    elem_size=elem,
                    elem_step=estep,
                    single_packet=False,
                )
                return g

            # ---------------- layer 1 ----------------
            GSZ = 4                 # L1 target tiles per PSUM bank
            for s, tiles in enumerate(plan.sts):
                groups = [tiles[i:i + GSZ] for i in range(0, len(tiles), GSZ)]
                # (group_idx, region, t, q, k) sequence in program order per
                # group, to place start/stop on bank-first/last matmuls
                seqs = [[] for _ in groups]
                for q in range(NCH):
                    for gi, grp in enumerate(groups):
                        for r, t in enumerate(grp):
                            for k in range(K4[t][q]):
                                seqs[gi].append((q, r, t, k))
                psG = [ps_scat.tile([P, len(grp) * P], FP, tag="scat",
                                    name=f"psG_{s}_{gi}")
                       for gi, grp in enumerate(groups)]
                done = [0] * len(groups)
                for q in range(NCH):
                    g = gather_supertile(idx1w, xsrc[q::NCH, :], CI, CI * NCH,
                                         s, q, "g1")
                    for gi, grp in enumerate(groups):
                        for r, t in enumerate(grp):
                            K = K4[t][q]
                            if K == 0:
                                continue
                            sOH = build_onehot_tc(t, q)
                            bb = plan.base_blk[s][(t, q)]
                            for k in range(K):
                                nc.tensor.matmul(
                                    out=psG[gi][:, r * P:(r + 1) * P],
                                    lhsT=g[:, (bb + k) * CI:(bb + k + 1) * CI],
                                    rhs=sOH[:, k * P:(k + 1) * P],
                                    start=(done[gi] == 0),
                                    stop=(done[gi] == len(seqs[gi]) - 1),
                                )
                                done[gi] += 1
                # epilogue per group then per tile
                for gi, grp in enumerate(groups):
                    W = len(grp) * P
                    t0 = grp[0]
                    db = ps_aux.tile([P, W], FP, tag="aux")
                    nc.tensor.matmul(
                        out=db[:], lhsT=ones_sb[:],
                        rhs=drow_sb[:, t0 * P:t0 * P + W],
                        start=True, stop=True)
                    db_sb = wk.tile([P, W], FP, tag="dbsb")
                    nc.scalar.activation(out=db_sb[:], in_=db[:],
                                         func=AF.Copy)
                    aggTn = wk.tile([P, W], FP, tag="aggTn")
                    nc.vector.tensor_tensor(out=aggTn[:], in0=psG[gi][:],
                                            in1=db_sb[:], op=OP.mult)
                    for r, t in enumerate(grp):
                        tb = slice(t * P, (t + 1) * P)
                        x_t = xp.tile([P, CI], FP, tag="x")
                        nc.sync.dma_start(out=x_t[:], in_=xloc[tb, :])
                        xT = ps_aux.tile([P, P], FP, tag="aux")
                        nc.tensor.transpose(out=xT[:], in_=x_t[:],
                                            identity=ident_sb[:])
                        xT_sb = wk.tile([P, P], FP, tag="xTsb")
                        nc.vector.tensor_copy(out=xT_sb[:], in_=xT[:])

                        o1 = ps_mm.tile([P, P], FP, tag="mm")
                        nc.tensor.matmul(out=o1[:], lhsT=w1o_sb[:],
                                         rhs=aggTn[:, r * P:(r + 1) * P],
                                         start=True, stop=False)
                        nc.tensor.matmul(out=o1[:], lhsT=w1r_sb[:],
                                         rhs=xT_sb[:],
                                         start=False, stop=True)
                        nc.scalar.activation(out=hT[:, tb], in_=o1[:],
                                             func=AF.Relu, bias=b1c_sb[:])

                        z2p = ps_aux.tile([P, CO], FP, tag="aux")
                        nc.tensor.matmul(out=z2p[:], lhsT=hT[:, tb],
                                         rhs=w2o_sb[:], start=True, stop=True)
                        z2sb = wk.tile([P, CO], FP, tag="z2sb")
                        nc.vector.tensor_copy(out=z2sb[:], in_=z2p[:])
                        nc.sync.dma_start(out=z2l[tb, :], in_=z2sb[:])

            # ---------------- allgather z2 ----------------
            nc.gpsimd.collective_compute(
                "AllGather", mybir.AluOpType.bypass,
                replica_groups=[list(range(cfg.CORES))],
                ins=[z2l.ap().opt()],
                outs=[z2f.ap().opt()],
            )

            # ---------------- layer 2 ----------------
            for s, tiles in enumerate(plan.sts):
                seq = []
                for q in range(NCH):
                    for t in tiles:
                        for k in range(K4[t][q]):
                            seq.append((q, t, k))
                psG = ps_scat.tile([P, len(tiles) * CO], FP, tag="scat")
                done = 0
                for q in range(NCH):
                    g = gather_supertile(idx2w, z2f[q::NCH, :], CO, CO * NCH,
                                         s, q, "g2")
                    for r, t in enumerate(tiles):
                        K = K4[t][q]
                        if K == 0:
                            continue
                        sOH = build_onehot_tc(t, q)
                        bb = plan.base_blk[s][(t, q)]
                        for k in range(K):
                            nc.tensor.matmul(
                                out=psG[:, r * CO:(r + 1) * CO],
                                lhsT=sOH[:, k * P:(k + 1) * P],
                                rhs=g[:, (bb + k) * CO:(bb + k + 1) * CO],
                                start=(done == 0),
                                stop=(done == len(seq) - 1),
                            )
                            done += 1
                for r, t in enumerate(tiles):
                    tb = slice(t * P, (t + 1) * P)
                    agg2n = wk.tile([P, CO], FP, tag="agg2n")
                    nc.vector.tensor_scalar(
                        out=agg2n[:], in0=psG[:, r * CO:(r + 1) * CO],
                        scalar1=dcol_sb[:, t:t + 1], scalar2=None,
                        op0=OP.mult)

                    rb = ps_mm.tile([P, CO], FP, tag="mm")
                    nc.tensor.matmul(out=rb[:], lhsT=hT[:, tb], rhs=w2r_sb[:],
                                     start=True, stop=False)
                    nc.tensor.matmul(out=rb[:], lhsT=ones_sb[:], rhs=b2r_sb[:],
                                     start=False, stop=True)

                    osb = wk.tile([P, CO], FP, tag="osb")
                    nc.vector.tensor_tensor(out=osb[:], in0=agg2n[:],
                                            in1=rb[:], op=OP.add)
                    nc.sync.dma_start(out=out[tb, :], in_=osb[:])

    nc.compile()
    return nc


def kernel(x, edge_index, W1_out, b1_out, W1_root, W2_out, b2_out, W2_root):
    from concourse import bass2jax

    cfg = Cfg()
    in_maps, plan, _ = preprocess(
        cfg, x, edge_index, W1_out, b1_out, W1_root, W2_out, b2_out, W2_root)
    nc = build_program(cfg, plan)
    results = bass2jax.run_bass_via_pjrt(nc, in_maps, n_cores=cfg.CORES)
    outs = [results[cc]["out"][:cfg.NS] for cc in range(cfg.CORES)]
    return np.concatenate(outs, axis=0).astype(np.float32)



# revision 48
# speedup vs baseline: 304.7738x; 304.7738x over previous
"""ClusterGCN 2-layer kernel for 8 Trainium2 NeuronCores (Bass/Tile).

Strategy (graph/data parallel, per the sharding hint):
  - Target nodes sharded 8 ways (12500/core, padded to 12544 = 98*128 tiles).
  - Each core owns the edges whose target (col) is in its shard, grouped by
    128-node target tile and source chunk (src%4), self loops as explicit
    edges, counts padded to be uniform across cores (single SPMD program).
  - Layer 1: x is gathered in bf16 (256B rows) via dma_gather per
    (supertile, chunk); scatter-add runs as one-hot matmuls into PSUM
    [feat, tgt]; one-hot blocks are built with per-partition-scalar
    is_equal (DVE 4x mode). deg_inv is applied via a rank-1 PE broadcast,
    then W1_out / W1_root (bf16) + bias + ReLU leave hT bf16 in SBUF.
    The root term uses a host-pretransposed x shard (no PE transposes).
  - z2 = relu(h) @ W2_out.T is accumulated bf16 in SBUF, written back in
    chunks under layer 1, and shipped with ONE AllGather (a single big
    transfer rides the higher-bandwidth tier and pays the fixed collective
    cost once; chunked/strided collective outputs are also rejected by the
    BIR verifier).
  - Layer 2 gathers z2 node PAIRS (256B bf16 rows) from the gathered table,
    one-hot matmuls into [tgt, 64] PSUM, applies deg_inv (Act scale) /
    root / bias and accumulates the output in SBUF, flushed in two DMAs.
  - Host-side greedy placement balances targets across (core, tile) bins
    (cap 512 edges per chunk cell, slot%4 == id%4) so K4 hits the 4-block
    minimum: 204800 gather rows/core/layer instead of 250112.

All index tables / transposed weights / iota are precomputed on the host.
"""
import math
import numpy as np
import ml_dtypes

BF = ml_dtypes.bfloat16
P = 128
NCH = 4          # gather-source chunks (int16 index limit)
ST = 12          # nominal target tiles per gather supertile
# tapered supertile sizes: small tail supertiles shorten the pipeline
# drain between the last gather and the AllGather
ST_SIZES = [12] * 7 + [7, 7]
CC_TILES = [98]      # collective chunk tile boundaries (ends, cumulative)


class Cfg:
    def __init__(self, n=100000, e=1600000, cores=8, c_in=128, c_hid=128,
                 c_out=64, st=ST):
        self.N, self.E, self.CORES = n, e, cores
        self.C_IN, self.C_HID, self.C_OUT = c_in, c_hid, c_out
        self.NS = n // cores                      # nodes per shard
        assert self.NS * cores == n
        assert self.NS % NCH == 0
        self.T = math.ceil(self.NS / P)           # target tiles per core
        self.NSP = self.T * P                     # padded shard size
        assert self.NSP % NCH == 0
        assert n % NCH == 0
        self.ST = st
        assert sum(ST_SIZES) == self.T
        self.NST = len(ST_SIZES)                  # supertiles


class Plan:
    """Static (cross-core-uniform) edge layout."""
    def __init__(self, cfg, K4):
        self.K4 = K4                              # [T][NCH] blocks
        self.Ktot = [sum(K4[t]) for t in range(cfg.T)]
        self.KSUM = sum(self.Ktot)
        self.toff = np.concatenate([[0], np.cumsum(self.Ktot)]).astype(int)
        # supertile structure (tapered sizes)
        self.sts = []
        t0_ = 0
        for sz in ST_SIZES:
            self.sts.append(list(range(t0_, t0_ + sz)))
            t0_ += sz
        # per (st, ch): NI and block base per tile
        self.NI = [[sum(K4[t][ch] for t in tiles) * P for ch in range(NCH)]
                   for tiles in self.sts]
        self.base_blk = []
        for tiles in self.sts:
            bb = {}
            for ch in range(NCH):
                acc = 0
                for t in tiles:
                    bb[(t, ch)] = acc
                    acc += K4[t][ch]
            self.base_blk.append(bb)
        # idx tensor column offsets per (st, ch), in int16 columns (NI/16)
        w = []
        for s in range(len(self.sts)):
            for ch in range(NCH):
                w.append(self.NI[s][ch] // 16)
        self.woff = np.concatenate([[0], np.cumsum(w)]).astype(int)
        self.WTOT = int(self.woff[-1])

    def wslice(self, s, ch):
        i = s * NCH + ch
        return int(self.woff[i]), int(self.woff[i + 1])


def wrap_idxs(flat):
    """[NI] int -> [128, NI//16] int16: j -> (j%16, j//16), replicated x8."""
    ni = flat.shape[0]
    w = flat.reshape(ni // 16, 16).T.astype(np.int16)
    return np.tile(w, (8, 1))


def preprocess(cfg, x, edge_index, W1_out, b1_out, W1_root, W2_out, b2_out,
               W2_root):
    N, NS, NSP, T = cfg.N, cfg.NS, cfg.NSP, cfg.T

    row = np.asarray(edge_index[0], dtype=np.int64)
    col = np.asarray(edge_index[1], dtype=np.int64)
    keep = row != col
    r = row[keep].astype(np.int64)
    c = col[keep].astype(np.int64)

    deg = np.bincount(c, minlength=N).astype(np.float32) + 1.0
    deg_inv = (1.0 / deg).astype(np.float32)
    # self loops are NOT materialized as edges; the kernel adds the self
    # term via identity matmuls into the PSUM accumulators.

    # --- balanced target placement ------------------------------------
    # Assign each target node a position (core, tile, slot) so that every
    # (bin, chunk) cell stays at/below CAP=4*128 edges: K4 then hits the
    # 4-block minimum almost everywhere instead of ceil-ing to 5.
    # Constraint: slot%4 == id%4 so the layer-1 (src%4) and layer-2
    # (position%4) chunk classes coincide (32 slots per residue per tile).
    nbins = cfg.CORES * T
    CAP = 4 * P
    d4 = np.bincount(c * NCH + (r % NCH), minlength=N * NCH
                     ).reshape(N, NCH).astype(np.int32)
    orderN = np.argsort(-d4.sum(1), kind="stable")
    cnt_b = np.zeros((nbins, NCH), np.int32)
    res_free = np.full((nbins, NCH), P // NCH, np.int32)
    fill = np.zeros((nbins, NCH), np.int32)
    pos_of = np.empty(N, np.int64)
    binid = np.arange(nbins)
    for node in orderN:
        rho = int(node % NCH)
        free = res_free[:, rho] > 0
        proj = cnt_b[free] + d4[node]
        viol = np.clip(proj - CAP, 0, None).sum(1)
        score = viol.astype(np.int64) * (1 << 20) + proj.max(1)
        b = int(binid[free][int(np.argmin(score))])
        cnt_b[b] += d4[node]
        res_free[b, rho] -= 1
        slot = fill[b, rho] * NCH + rho
        fill[b, rho] += 1
        core, tile = divmod(b, T)
        pos_of[node] = core * NSP + tile * P + slot

    tgt_pos = pos_of[c]
    shard = tgt_pos // NSP
    lt = tgt_pos - shard * NSP
    tt = lt // P
    cl = lt % P
    ch = r % NCH

    # per (core, tile, chunk) counts -> uniform K4
    flat_key = (shard * T + tt) * NCH + ch
    cnt = np.bincount(flat_key, minlength=cfg.CORES * T * NCH
                      ).reshape(cfg.CORES, T, NCH)
    K4 = [[int(math.ceil(cnt[:, t, q].max() / P)) for q in range(NCH)]
          for t in range(T)]
    plan = Plan(cfg, K4)

    order = np.lexsort((ch, tt, shard))
    r_s, tt_s, cl_s, ch_s = r[order], tt[order], cl[order], ch[order]
    key_s = (shard[order] * T + tt_s) * NCH + ch_s
    seg = np.searchsorted(key_s, np.arange(cfg.CORES * T * NCH + 1))

    iota = np.broadcast_to(np.arange(P, dtype=BF), (P, P)).copy()
    ident = np.eye(P, dtype=BF)
    ones = np.ones((1, P), np.float32)
    ones_bf = np.ones((1, P), BF)
    w1o = np.ascontiguousarray(np.asarray(W1_out, np.float32).T.astype(BF))
    w1r = np.ascontiguousarray(np.asarray(W1_root, np.float32).T.astype(BF))
    w2o = np.ascontiguousarray(np.asarray(W2_out, np.float32).T.astype(BF))
    w2r = np.ascontiguousarray(np.asarray(W2_root, np.float32).T.astype(BF))
    b1c = np.asarray(b1_out, np.float32).reshape(-1, 1)
    b2r = np.asarray(b2_out, np.float32).reshape(1, -1).astype(BF)
    xf = np.asarray(x, np.float32)
    xb = np.ascontiguousarray(xf.astype(BF))

    in_maps = []
    for cc in range(cfg.CORES):
        colv = np.full((P, plan.KSUM), -1.0, np.float32)
        srcg = np.zeros((P, plan.KSUM), np.int64)
        for t in range(T):
            coff = int(plan.toff[t])
            ccol = 0
            for q in range(NCH):
                i0 = seg[(cc * T + t) * NCH + q]
                i1 = seg[(cc * T + t) * NCH + q + 1]
                K = K4[t][q]
                L = i1 - i0
                s_pad = np.full(K * P, q, np.int64)    # pad: chunk row 0
                c_pad = np.full(K * P, -1.0, np.float32)
                s_pad[:L] = r_s[i0:i1]
                c_pad[:L] = cl_s[i0:i1].astype(np.float32)
                srcg[:, coff + ccol:coff + ccol + K] = \
                    s_pad.reshape(K, P).T
                colv[:, coff + ccol:coff + ccol + K] = \
                    c_pad.reshape(K, P).T
                ccol += K
        # build wrapped idx tensors per (st, ch) call.
        # z2f is laid out CHUNK-MAJOR (each AllGather chunk's 8-core block
        # is contiguous): zrow maps position -> chunk-major row.
        def pos_to_zrow(p):
            core, l = p // NSP, p % NSP
            t = l // P
            zr = np.zeros_like(p)
            cbase = 0
            prev = 0
            for ce in CC_TILES:
                nt = ce - prev
                m = (t >= prev) & (t < ce)
                zr[m] = (cbase + core[m] * nt * P + l[m] - prev * P)
                cbase += cfg.CORES * nt * P
                prev = ce
            return zr
        zrow = pos_to_zrow(pos_of[srcg])              # chunk-major z2 row
        idx1w = np.zeros((P, plan.WTOT), np.int16)
        idx2w = np.zeros((P, plan.WTOT), np.int16)
        for s, tiles in enumerate(plan.sts):
            for q in range(NCH):
                w0, w1 = plan.wslice(s, q)
                if w1 == w0:
                    continue
                flat1 = np.zeros((w1 - w0) * 16, np.int64)
                flat2 = np.zeros((w1 - w0) * 16, np.int64)
                pos = 0
                for t in tiles:
                    coff = int(plan.toff[t])
                    cbase = sum(K4[t][qq] for qq in range(q))
                    K = K4[t][q]
                    for k in range(K):
                        col_i = coff + cbase + k
                        flat1[pos:pos + P] = srcg[:, col_i] // NCH
                        flat2[pos:pos + P] = zrow[:, col_i] // NCH
                        pos += P
                assert pos == (w1 - w0) * 16
                idx1w[:, w0:w1] = wrap_idxs(flat1)
                idx2w[:, w0:w1] = wrap_idxs(flat2)

        xloc = np.zeros((NSP, cfg.C_IN), np.float32)
        dloc = np.ones(NSP, np.float32)
        mine = (pos_of >= cc * NSP) & (pos_of < (cc + 1) * NSP)
        nodes_c = np.nonzero(mine)[0]
        loc = pos_of[nodes_c] - cc * NSP
        xloc[loc] = xf[nodes_c]
        dloc[loc] = deg_inv[nodes_c]
        xlocT = np.ascontiguousarray(xloc.T.astype(BF))   # [C_IN, NSP]
        dcol = np.ascontiguousarray(dloc.reshape(T, P).T)
        drow = dloc.reshape(1, NSP)

        in_maps.append({
            "xsrc": xb, "xlocT": xlocT,
            "idx1w": idx1w, "idx2w": idx2w, "colv": colv,
            "dcol": dcol, "drow": drow,
            "w1o": w1o, "w1r": w1r, "w2o": w2o, "w2r": w2r,
            "b1c": b1c, "b2r": b2r, "ones": ones, "ones_bf": ones_bf,
            "iota": iota, "ident": ident,
        })
    return in_maps, plan, pos_of


def build_program(cfg, plan):
    import concourse.bass as bass  # noqa: F401
    import concourse.bacc as bacc
    import concourse.mybir as mybir
    import concourse.tile as tile

    FP = mybir.dt.float32
    BF16 = mybir.dt.bfloat16
    I16 = mybir.dt.int16
    AF = mybir.ActivationFunctionType
    OP = mybir.AluOpType
    N, NSP, T = cfg.N, cfg.NSP, cfg.T
    CI, CH_, CO = cfg.C_IN, cfg.C_HID, cfg.C_OUT
    K4, KSUM = plan.K4, plan.KSUM
    GSZ = 4                   # L1 target tiles per PSUM bank
    GSZ2 = 6                  # L2 target tiles per PSUM bank
    # collective chunk boundaries: after these supertiles, ship z2 tiles
    # write-back chunks: z2 SBUF->DRAM DMAs overlapped under layer 1
    WB_BOUNDS = [2, 4, 6, cfg.NST - 1]
    assert WB_BOUNDS[-1] == cfg.NST - 1
    # a single AllGather (one big transfer rides the high-bw tier and pays
    # the fixed collective cost once); out AP must be contiguous
    assert CC_TILES == [T]

    nc = bacc.Bacc("TRN2", target_bir_lowering=False, debug=False,
                   num_devices=cfg.CORES)

    xsrc = nc.dram_tensor("xsrc", [N, CI], BF16, kind="ExternalInput")
    xlocT = nc.dram_tensor("xlocT", [CI, NSP], BF16, kind="ExternalInput")
    idx1w = nc.dram_tensor("idx1w", [P, plan.WTOT], I16, kind="ExternalInput")
    idx2w = nc.dram_tensor("idx2w", [P, plan.WTOT], I16, kind="ExternalInput")
    colv = nc.dram_tensor("colv", [P, KSUM], FP, kind="ExternalInput")
    dcol = nc.dram_tensor("dcol", [P, T], FP, kind="ExternalInput")
    drow = nc.dram_tensor("drow", [1, NSP], FP, kind="ExternalInput")
    w1o = nc.dram_tensor("w1o", [CI, CH_], BF16, kind="ExternalInput")
    w1r = nc.dram_tensor("w1r", [CI, CH_], BF16, kind="ExternalInput")
    w2o = nc.dram_tensor("w2o", [CH_, CO], BF16, kind="ExternalInput")
    w2r = nc.dram_tensor("w2r", [CH_, CO], BF16, kind="ExternalInput")
    b1c = nc.dram_tensor("b1c", [CH_, 1], FP, kind="ExternalInput")
    b2r = nc.dram_tensor("b2r", [1, CO], BF16, kind="ExternalInput")
    ones = nc.dram_tensor("ones", [1, P], FP, kind="ExternalInput")
    ones_bf = nc.dram_tensor("ones_bf", [1, P], BF16, kind="ExternalInput")
    iota = nc.dram_tensor("iota", [P, P], BF16, kind="ExternalInput")
    ident = nc.dram_tensor("ident", [P, P], BF16, kind="ExternalInput")

    # out is [feat-tile-flat]: out[p, t*CO+c] = result[t*128+p, c]; the
    # host reshapes. Written in two chunks so only the tail DMA is exposed.
    out = nc.dram_tensor("out", [P, T * CO], FP, kind="ExternalOutput")
    z2l = nc.dram_tensor("z2l", [NSP, CO], BF16)
    z2f = nc.dram_tensor("z2f", [cfg.CORES * NSP, CO], BF16,
                         addr_space="Shared")
    # pair-row view of z2f: [TOT//2, 2*CO]; chunked by pair parity
    z2fp = z2f[:, :].rearrange("(r two) c -> r (two c)", two=2)

    with tile.TileContext(nc) as tc:
        with (
            tc.tile_pool(name="cst", bufs=1) as cst,
            tc.tile_pool(name="hp", bufs=1) as hp,
            tc.tile_pool(name="gp", bufs=3) as gp,
            tc.tile_pool(name="ip", bufs=9) as ip,
            tc.tile_pool(name="sp", bufs=12) as sp,
            tc.tile_pool(name="wk", bufs=3) as wk,
            tc.tile_pool(name="ps_scat", bufs=4, space="PSUM") as ps_scat,
            tc.tile_pool(name="ps_mm", bufs=2, space="PSUM") as ps_mm,
            tc.tile_pool(name="ps_aux", bufs=2, space="PSUM") as ps_aux,
        ):
            def load_const(t_dram, shape, dtype=FP):
                t_sb = cst.tile(shape, dtype, tag=t_dram.name)
                nc.sync.dma_start(out=t_sb[:], in_=t_dram[:, :])
                return t_sb

            # prefetch the first two supertiles' gather indices ahead of
            # the const loads so the Pool gather stream starts immediately
            pre_idx = {}
            for s_ in range(2):
                for q in range(NCH):
                    w0, w1 = plan.wslice(s_, q)
                    it = ip.tile([P, w1 - w0], I16, tag="idx",
                                 name=f"pre{s_}_{q}")
                    nc.sync.dma_start(out=it[:], in_=idx1w[:, w0:w1])
                    pre_idx[(s_, q)] = it

            # one-hot inputs first: they gate the first scatter matmuls;
            # xlocT next: it closes the first PSUM accumulation chain
            iota_sb = load_const(iota, [P, P], BF16)
            colv_sb = load_const(colv, [P, KSUM])
            ident_sb = load_const(ident, [P, P], BF16)
            xlocT_sb = load_const(xlocT, [CI, NSP], BF16)
            ones_sb = load_const(ones, [1, P])
            drow_sb = load_const(drow, [1, NSP])
            w1o_sb = load_const(w1o, [CI, CH_], BF16)
            w1r_sb = load_const(w1r, [CI, CH_], BF16)
            w2o_sb = load_const(w2o, [CH_, CO], BF16)
            w2r_sb = load_const(w2r, [CH_, CO], BF16)
            b1c_sb = load_const(b1c, [CH_, 1])
            b2r_sb = load_const(b2r, [1, CO], BF16)
            onbf_sb = load_const(ones_bf, [1, P], BF16)
            dcol_sb = load_const(dcol, [P, T])

            hT = hp.tile([P, NSP], BF16, tag="hT")         # [feat, tgt]
            outsb = hp.tile([P, T * CO], FP, tag="outsb")  # [tgt, t*CO]
            # z2 accumulators, one tile per collective chunk
            ccs = [(0 if i == 0 else plan.sts[WB_BOUNDS[i - 1]][-1] + 1,
                    plan.sts[WB_BOUNDS[i]][-1] + 1)
                   for i in range(len(WB_BOUNDS))]
            z2c = [hp.tile([P, (b - a) * CO], BF16, name=f"z2c{i}",
                           tag=f"z2c{i}")
                   for i, (a, b) in enumerate(ccs)]

            def tile_chunk(t):
                for i, (a, b) in enumerate(ccs):
                    if a <= t < b:
                        return i, t - a
                raise AssertionError

            def build_onehot(t, q, k):
                """One-hot [slot, tgt] for block k of (tile t, chunk q)."""
                cbase = sum(K4[t][qq] for qq in range(q))
                o = int(plan.toff[t]) + cbase + k
                s = sp.tile([P, P], BF16, tag="s")
                nc.vector.tensor_scalar(
                    out=s[:], in0=iota_sb[:], scalar1=colv_sb[:, o:o + 1],
                    scalar2=None, op0=OP.is_equal)
                return s

            def gather_supertile(idxw_dram, table_ap, elem, estep, s, q, tag,
                                 dtype=BF16):
                NI = plan.NI[s][q]
                if NI == 0:
                    return None
                if tag == "g1" and s <= 1:
                    it = pre_idx[(s, q)]
                else:
                    w0, w1 = plan.wslice(s, q)
                    it = ip.tile([P, w1 - w0], I16, tag="idx", name=tag + "i")
                    nc.sync.dma_start(out=it[:], in_=idxw_dram[:, w0:w1])
                g = gp.tile([P, (NI // P) * elem], dtype, tag="g", name=tag)
                nc.gpsimd.dma_gather(
                    out_ap=g[:].rearrange("p (k j) -> p k j", k=NI // P),
                    in_ap=table_ap,
                    idxs_ap=it[:],
                    num_idxs=NI,
                    num_idxs_reg=NI,
                    elem_size=elem,
                    elem_step=estep,
                    single_packet=False,
                )
                return g

            # ---------------- layer 1 ----------------
            for s, tiles in enumerate(plan.sts):
                groups = [tiles[i:i + GSZ] for i in range(0, len(tiles), GSZ)]
                seqs = [[] for _ in groups]
                for q in range(NCH):
                    for gi, grp in enumerate(groups):
                        for r, t in enumerate(grp):
                            for k in range(K4[t][q]):
                                seqs[gi].append((q, r, t, k))
                psG = [ps_scat.tile([P, len(grp) * P], FP, tag="scat",
                                    name=f"psG_{s}_{gi}")
                       for gi, grp in enumerate(groups)]
                done = [0] * len(groups)
                tot = [len(sq) + len(grp)
                       for sq, grp in zip(seqs, groups)]
                for q in range(NCH):
                    g = gather_supertile(idx1w, xsrc[q::NCH, :], CI, CI * NCH,
                                         s, q, "g1")
                    for gi, grp in enumerate(groups):
                        for r, t in enumerate(grp):
                            K = K4[t][q]
                            bb = plan.base_blk[s][(t, q)]
                            for k in range(K):
                                sOH = build_onehot(t, q, k)
                                nc.tensor.matmul(
                                    out=psG[gi][:, r * P:(r + 1) * P],
                                    lhsT=g[:, (bb + k) * CI:(bb + k + 1) * CI],
                                    rhs=sOH[:],
                                    start=(done[gi] == 0),
                                    stop=(done[gi] == tot[gi] - 1),
                                )
                                done[gi] += 1
                # self-loop term: psG[:, tile] += x_tile (identity matmul)
                for gi, grp in enumerate(groups):
                    for r, t in enumerate(grp):
                        tb = slice(t * P, (t + 1) * P)
                        nc.tensor.matmul(
                            out=psG[gi][:, r * P:(r + 1) * P],
                            lhsT=ident_sb[:], rhs=xlocT_sb[:, tb],
                            start=(done[gi] == 0),
                            stop=(done[gi] == tot[gi] - 1),
                        )
                        done[gi] += 1
                # epilogue per group then per tile
                for gi, grp in enumerate(groups):
                    W = len(grp) * P
                    t0 = grp[0]
                    db = ps_aux.tile([P, W], FP, tag="aux")
                    nc.tensor.matmul(
                        out=db[:], lhsT=ones_sb[:],
                        rhs=drow_sb[:, t0 * P:t0 * P + W],
                        start=True, stop=True)
                    db_sb = wk.tile([P, W], FP, tag="dbsb")
                    nc.scalar.activation(out=db_sb[:], in_=db[:],
                                         func=AF.Copy)
                    aggTn = wk.tile([P, W], BF16, tag="aggTn")
                    nc.vector.tensor_tensor(out=aggTn[:], in0=psG[gi][:],
                                            in1=db_sb[:], op=OP.mult)
                    for r, t in enumerate(grp):
                        tb = slice(t * P, (t + 1) * P)
                        o1 = ps_mm.tile([P, P], FP, tag="mm")
                        nc.tensor.matmul(out=o1[:], lhsT=w1o_sb[:],
                                         rhs=aggTn[:, r * P:(r + 1) * P],
                                         start=True, stop=False)
                        nc.tensor.matmul(out=o1[:], lhsT=w1r_sb[:],
                                         rhs=xlocT_sb[:, tb],
                                         start=False, stop=True)
                        nc.scalar.activation(out=hT[:, tb], in_=o1[:],
                                             func=AF.Relu, bias=b1c_sb[:])

                        z2p = ps_aux.tile([P, CO], FP, tag="aux")
                        nc.tensor.matmul(out=z2p[:], lhsT=hT[:, tb],
                                         rhs=w2o_sb[:], start=True, stop=True)
                        ci, r2 = tile_chunk(t)
                        nc.scalar.activation(
                            out=z2c[ci][:, r2 * CO:(r2 + 1) * CO],
                            in_=z2p[:], func=AF.Copy)
                # ship the z2 chunk as soon as it completes (SP engine)
                for i, bound in enumerate(WB_BOUNDS):
                    if s == bound:
                        a, b = ccs[i]
                        nt = b - a
                        nc.sync.dma_start(
                            out=z2l[a * P:b * P, :].rearrange(
                                "(t p) c -> p t c", p=P),
                            in_=z2c[i][:].rearrange(
                                "p (t c) -> p t c", t=nt))

            # single AllGather AFTER the whole L1 gather stream in Pool
            # program order (contiguous in and out APs)
            nc.gpsimd.collective_compute(
                "AllGather", mybir.AluOpType.bypass,
                replica_groups=[list(range(cfg.CORES))],
                ins=[z2l[:, :].opt()],
                outs=[z2f[:, :].opt()],
            )

            # ---------------- layer 2 root/bias (collective-independent) ---
            for t in range(T):
                tb = slice(t * P, (t + 1) * P)
                rb = ps_mm.tile([P, CO], FP, tag="mm")
                nc.tensor.matmul(out=rb[:], lhsT=hT[:, tb], rhs=w2r_sb[:],
                                 start=True, stop=False)
                nc.tensor.matmul(out=rb[:], lhsT=onbf_sb[:], rhs=b2r_sb[:],
                                 start=False, stop=True)
                nc.scalar.activation(out=outsb[:, t * CO:(t + 1) * CO],
                                     in_=rb[:], func=AF.Copy)

            # ---------------- layer 2 scatter ----------------
            for s, tiles in enumerate(plan.sts):
                subs = [tiles[i:i + GSZ2]
                        for i in range(0, len(tiles), GSZ2)]
                seqs = [[] for _ in subs]
                for q in range(NCH):
                    for si, sub in enumerate(subs):
                        for t in sub:
                            for k in range(K4[t][q]):
                                seqs[si].append((q, t, k))
                psGs = [ps_scat.tile([P, len(sub) * CO], FP, tag="scat",
                                     name=f"psG2_{s}_{si}")
                        for si, sub in enumerate(subs)]
                done = [0] * len(subs)
                tot = [len(sq) + len(sub) for sq, sub in zip(seqs, subs)]
                for q in range(NCH):
                    # pair table: rows of z2fp with parity q>>1; the half
                    # within the pair is q&1 (zrow%4 == src%4 == q).
                    g = gather_supertile(idx2w, z2fp[(q >> 1)::2, :],
                                         2 * CO, 4 * CO, s, q, "g2")
                    h = (q & 1) * CO
                    for si, sub in enumerate(subs):
                        for r, t in enumerate(sub):
                            K = K4[t][q]
                            bb = plan.base_blk[s][(t, q)]
                            for k in range(K):
                                sOH = build_onehot(t, q, k)
                                nc.tensor.matmul(
                                    out=psGs[si][:, r * CO:(r + 1) * CO],
                                    lhsT=sOH[:],
                                    rhs=g[:, (bb + k) * 2 * CO + h:
                                          (bb + k) * 2 * CO + h + CO],
                                    start=(done[si] == 0),
                                    stop=(done[si] == tot[si] - 1),
                                )
                                done[si] += 1
                # self-loop term: psG2[:, tile] += z2_tile (identity matmul)
                for si, sub in enumerate(subs):
                    for r, t in enumerate(sub):
                        ci, r2 = tile_chunk(t)
                        nc.tensor.matmul(
                            out=psGs[si][:, r * CO:(r + 1) * CO],
                            lhsT=ident_sb[:],
                            rhs=z2c[ci][:, r2 * CO:(r2 + 1) * CO],
                            start=(done[si] == 0),
                            stop=(done[si] == tot[si] - 1),
                        )
                        done[si] += 1
                for si, sub in enumerate(subs):
                    for r, t in enumerate(sub):
                        agg2n = wk.tile([P, CO], FP, tag="agg2n")
                        nc.scalar.activation(
                            out=agg2n[:], in_=psGs[si][:, r * CO:(r + 1) * CO],
                            func=AF.Copy, scale=dcol_sb[:, t:t + 1])
                        nc.vector.tensor_tensor(
                            out=outsb[:, t * CO:(t + 1) * CO],
                            in0=outsb[:, t * CO:(t + 1) * CO],
                            in1=agg2n[:], op=OP.add)
                # flush finished output columns chunk-wise so only the last
                # small DMA is exposed at the kernel tail
                for i, bound in enumerate(WB_BOUNDS):
                    if s == bound:
                        a, b = ccs[i]
                        nc.sync.dma_start(
                            out=out[:, a * CO:b * CO],
                            in_=outsb[:, a * CO:b * CO])

    nc.compile()
    return nc


def kernel(x, edge_index, W1_out, b1_out, W1_root, W2_out, b2_out, W2_root):
    from concourse import bass2jax

    cfg = Cfg()
    in_maps, plan, pos_of = preprocess(
        cfg, x, edge_index, W1_out, b1_out, W1_root, W2_out, b2_out, W2_root)
    nc = build_program(cfg, plan)
    results = bass2jax.run_bass_via_pjrt(nc, in_maps, n_cores=cfg.CORES)
    allout = np.concatenate([
        results[cc]["out"].reshape(128, cfg.T, cfg.C_OUT)
        .transpose(1, 0, 2).reshape(cfg.NSP, cfg.C_OUT)
        for cc in range(cfg.CORES)
    ], axis=0)
    return allout[pos_of].astype(np.float32)
